# revision 74
# baseline (speedup 1.0000x reference)
"""Trainium2 Bass kernel for nn_MultiHeadedLinrec (linear attention).

Math (per batch element, reference semantics):
    q = elu(x_q @ Wq.T + bq)    [S, E] viewed as [S, H, d]
    k = elu(x_k @ Wk.T + bk)
    v = x_v @ Wv.T + bv
    k <- k / (||k||_seq * sqrt(S))     (per (h, d) column norm over S)
    q <- q / (||q||_d   * sqrt(d))     (per (s, h) row norm over d)
    scores_h = k_h^T @ v_h             [d, d]
    out = concat_h(q_h @ scores_h) @ Wo.T + bo

Kernel strategy (one NeuronCore per batch element, 8 cores data-parallel):
  Host pre-transposes x_q/x_k/x_v to [E, S] and converts x + weights to
  bf16, so the device never runs PE transposes and every matmul streams
  bf16 moving operands at 1 cyc/row (fp32 PSUM accumulation throughout).

  Phase A (stream S in 512-col blocks, 128-row subtiles): project k and v
    into natural [s, e] layout (data chunks stationary, weight chunks
    moving), ELU(k), pack per-head [v|k] bf16 tiles, accumulate per-head
    scoresT = v_h^T k_h and the k Gram matrix on the PE (scores matmuls are
    emitted with a one-subtile lag so the in-order PE queue never waits on
    the ELU chain).
  Phase B: knorm from the Gram diagonal (DMA diag gather), then fold
    k-norm + scoresT + Wo into one fused weight
    W2[i, o] = (scores @ Wo.T)[i, o] / (knorm[i] * sqrt(S)), bf16.
  Phase C (stream S in 512-col blocks): q projection in transposed [e, s]
    layout (weight chunks stationary, data moving), ELU, per-head row
    norms via block-ones matmul + PE broadcast, then out = qn^T @ W2 + bo
    in natural layout.  B and C are software-pipelined: the W2 build and
    each block's inv-norm chain trail the projection stream by two blocks,
    so the in-order PE queue never waits on DVE/Act epilogues.  PSUM pools
    are created so late-needed pools take the banks phase A frees last.

This walrus build only supports ONE sync wait per instruction; Tile emits
multi-wait instructions, so we legalize the BIR JSON by hoisting extra waits
onto inserted NoOps (see _legalize_sync_json).
"""

import json

import ml_dtypes
import numpy as np

import concourse.bass as bass
import concourse.mybir as mybir
import concourse.tile as tile
from concourse.bass_utils import run_bass_kernel_spmd

dt = mybir.dt
AF = mybir.ActivationFunctionType
ALU = mybir.AluOpType

P = 128
E = 1024
H = 16
D = 64
N_CORES = 8
EC = E // P  # 8 chunks of 128 along the embedding dim
SBLK = 512  # s-block width
BF16 = ml_dtypes.bfloat16


# --------------------------------------------------------------------------
# BIR sync legalization: max one wait / one update per instruction.
# --------------------------------------------------------------------------
def _legalize_sync_json(bir_json: bytes) -> bytes:
    m = json.loads(bir_json)
    counter = [0]

    def fresh():
        counter[0] += 1
        return f"I-synclift-{counter[0]}"

    for f in m["functions"]:
        for blk in f["blocks"]:
            out = []
            for ins in blk["instructions"]:
                si = ins.get("sync_info")
                if not si:
                    out.append(ins)
                    continue
                waits = si.get("on_wait") or []
                updates = si.get("on_update") or []
                if len(waits) <= 1 and len(updates) <= 1:
                    out.append(ins)
                    continue
                eng = ins.get("engine")
                dbg = ins.get("debug")
                for w in waits[:-1]:
                    out.append(
                        {
                            "debug": dbg,
                            "engine": eng,
                            "ins": [],
                            "name": fresh(),
                            "opcode": "NoOp",
                            "outs": [],
                            "sync_info": {"on_update": [], "on_wait": [w]},
                        }
                    )
                si["on_wait"] = waits[-1:]
                post = [
                    {
                        "debug": dbg,
                        "engine": eng,
                        "ins": [],
                        "name": fresh(),
                        "opcode": "NoOp",
                        "outs": [],
                        "sync_info": {"on_update": [u], "on_wait": []},
                    }
                    for u in updates[1:]
                ]
                si["on_update"] = updates[:1]
                out.append(ins)
                out.extend(post)
            blk["instructions"] = out
    return json.dumps(m).encode()


def _patch_bass(nc):
    orig = nc.to_json_bytes

    def patched():
        return _legalize_sync_json(orig())

    nc.to_json_bytes = patched
    return nc


# --------------------------------------------------------------------------
# Kernel builder
# --------------------------------------------------------------------------
def build(S: int = 4096, with_bias: bool = False, cfg: dict | None = None):
    cfg = {"a_pj": 3, "a_kv": 3, "c_pj": 3, "c_fin": 2, **(cfg or {})}
    dbg = cfg.get("debug", False)
    ST = S // P  # number of 128-row s-tiles
    NBLK = S // SBLK
    JB = SBLK // P  # s-tiles per block (4)

    nc = bass.Bass(trn_type="TRN2", target_bir_lowering=False, debug=False)

    bf = dt.bfloat16
    f32 = dt.float32

    xqT = nc.dram_tensor("xqT", [E, S], bf, kind="ExternalInput").ap()
    xkT = nc.dram_tensor("xkT", [E, S], bf, kind="ExternalInput").ap()
    xvT = nc.dram_tensor("xvT", [E, S], bf, kind="ExternalInput").ap()
    WqTd = nc.dram_tensor("WqT", [E, E], bf, kind="ExternalInput").ap()
    WkTd = nc.dram_tensor("WkT", [E, E], bf, kind="ExternalInput").ap()
    WvTd = nc.dram_tensor("WvT", [E, E], bf, kind="ExternalInput").ap()
    WoTd = nc.dram_tensor("WoT", [E, E], bf, kind="ExternalInput").ap()
    if with_bias:
        bqr = nc.dram_tensor("bqr", [1, E], bf, kind="ExternalInput").ap()
        bkr = nc.dram_tensor("bkr", [1, E], bf, kind="ExternalInput").ap()
        bvr = nc.dram_tensor("bvr", [1, E], bf, kind="ExternalInput").ap()
        bor = nc.dram_tensor("bor", [1, E], bf, kind="ExternalInput").ap()
    bpickd = nc.dram_tensor("bpick", [H, EC * P], bf, kind="ExternalInput").ap()
    out = nc.dram_tensor("out", [S, E], f32, kind="ExternalOutput").ap()
    if dbg:
        dbg_kv = nc.dram_tensor("dbg_kv", [P, 2 * E], f32, kind="ExternalOutput").ap()
        dbg_gram = nc.dram_tensor("dbg_gram", [D, H * D], f32, kind="ExternalOutput").ap()
        dbg_invk = nc.dram_tensor("dbg_invk", [P, EC], f32, kind="ExternalOutput").ap()
        dbg_w2 = nc.dram_tensor("dbg_w2", [E, E], f32, kind="ExternalOutput").ap()
        dbg_qt = nc.dram_tensor("dbg_qt", [E, SBLK], f32, kind="ExternalOutput").ap()
        dbg_qn = nc.dram_tensor("dbg_qn", [E, SBLK], f32, kind="ExternalOutput").ap()

    with tile.TileContext(nc) as tc:
        with (
            tc.tile_pool(name="consts", bufs=1) as consts,
            tc.tile_pool(name="small", bufs=1) as small,
            tc.tile_pool(name="wts", bufs=1) as wts,
            tc.tile_pool(name="drpool", bufs=1, space="DRAM") as drpool,
            tc.tile_pool(name="dbgpool", bufs=1) as dbgpool,
        ):
            # ---------------- constants ----------------
            # zrow first: the scores-psum zeroing matmuls are the PE's first
            # instructions and should not wait on the other const memsets
            zrow = consts.tile([1, SBLK], bf, name="zrow")
            nc.vector.memset(zrow[:], 0.0)
            # blockones_ot [P, H]: col 2*ot -> partitions 0:64, col 2*ot+1 ->
            # partitions 64:128 (head-pair sum masks for qss accumulation)
            blockones = []
            for c in range(EC):
                t = consts.tile([P, H], bf, name=f"blockones_{c}")
                nc.vector.memset(t[:], 0.0)
                nc.vector.memset(t[0:D, 2 * c : 2 * c + 1], 1.0)
                nc.vector.memset(t[D:P, 2 * c + 1 : 2 * c + 2], 1.0)
                blockones.append(t)
            # blockpick_ot [H, P] = blockones_ot^T (broadcast masks); loaded
            # from a host constant because DVE memsets cannot start at odd
            # partitions.
            bpick_all = consts.tile([H, EC * P], bf, name="bpick_all")
            nc.sync.dma_start(bpick_all[:], bpickd)
            blockpick = [bpick_all[:, c * P : (c + 1) * P] for c in range(EC)]
            ones_row = None
            if with_bias:
                ones_row = consts.tile([1, SBLK], bf, name="ones_row")
                nc.vector.memset(ones_row[:], 1.0)

            # bd staging (scoresT block-diagonal per chunk), zeroed once
            bd_st = []
            for pr in range(EC):
                s_t = small.tile([P, P], f32, name=f"bd_st_{pr}")
                nc.vector.memset(s_t[:], 0.0)
                bd_st.append(s_t)

            # ---------------- weights (all bf16, straight DMA) -----------
            def load_wt(WTd, name):
                tiles = []
                for c in range(EC):
                    t = wts.tile([P, E], bf, name=f"{name}T_{c}")
                    nc.sync.dma_start(t[:], WTd[c * P : (c + 1) * P, :])
                    tiles.append(t)
                return tiles

            def load_brow(src, name):
                t = small.tile([1, E], bf, name=name)
                nc.sync.dma_start(t[:], src)
                return t

            bk_row = bv_row = bq_row = bo_row = None
            if with_bias:
                bk_row = load_brow(bkr, "bk_row")
                bv_row = load_brow(bvr, "bv_row")

            # ================= PHASE A ====================================
            c_in_scope = tc.tile_pool(name="c_in", bufs=2)
            c_in = c_in_scope.__enter__()
            a_in_scope = tc.tile_pool(name="a_in", bufs=2)
            a_in = a_in_scope.__enter__()

            def load_xq(bi):
                t = c_in.tile([P, EC * SBLK], bf, name="xq_blk")
                nc.sync.dma_start(
                    t[:].rearrange("p (c s) -> p c s", c=EC),
                    xqT[:, bi * SBLK : (bi + 1) * SBLK].rearrange(
                        "(c p) s -> p c s", p=P
                    ),
                )
                return t

            def load_xblk(src, bi, name):
                t = a_in.tile([P, EC * SBLK], bf, name=name)
                nc.sync.dma_start(
                    t[:].rearrange("p (c s) -> p c s", c=EC),
                    src[:, bi * SBLK : (bi + 1) * SBLK].rearrange(
                        "(c p) s -> p c s", p=P
                    ),
                )
                return t

            # Startup: interleave the k-weight and first-block loads per
            # chunk so the first projection matmuls start ~1us in (the DMA
            # device is serial; a monolithic 3MB prefix stalls the PE 12us)
            WkT = [wts.tile([P, E], bf, name=f"WkT_{c}") for c in range(EC)]
            WvT = [wts.tile([P, E], bf, name=f"WvT_{c}") for c in range(EC)]
            xk_cur = a_in.tile([P, EC * SBLK], bf, name="xk_blk")
            xv_cur = a_in.tile([P, EC * SBLK], bf, name="xv_blk")
            for c in range(EC):
                nc.sync.dma_start(WkT[c][:], WkTd[c * P : (c + 1) * P, :])
                nc.sync.dma_start(
                    xk_cur[:, c * SBLK : (c + 1) * SBLK],
                    xkT[c * P : (c + 1) * P, 0:SBLK],
                )
            for c in range(EC):
                nc.sync.dma_start(WvT[c][:], WvTd[c * P : (c + 1) * P, :])
                nc.sync.dma_start(
                    xv_cur[:, c * SBLK : (c + 1) * SBLK],
                    xvT[c * P : (c + 1) * P, 0:SBLK],
                )
            WqT = load_wt(WqTd, "Wq")
            WoT = load_wt(WoTd, "Wo")
            if with_bias:
                bq_row = load_brow(bqr, "bq_row")
                bo_row = load_brow(bor, "bo_row")

            a_sc_scope = tc.tile_pool(name="a_sc_ps", bufs=1, space="PSUM")
            a_sc_ps = a_sc_scope.__enter__()
            scores_ps = a_sc_ps.tile([P, H * D], f32, name="scores_ps")
            # Pre-zero via start=True matmuls: a matmul's start bit zeroes the
            # whole PSUM bank, so the per-head 64-col accumulation below must
            # never use start=True (it would erase sibling heads' partials).
            for hb in range(2):
                nc.tensor.matmul(
                    scores_ps[:, hb * SBLK : (hb + 1) * SBLK],
                    zrow[:, 0:P],
                    zrow[:],
                    start=True,
                    stop=True,
                    skip_group_check=True,
                )

            with (
                tc.tile_pool(name="a_pj_ps", bufs=cfg["a_pj"] * (2 if cfg.get("a_shared") else 1), space="PSUM") as a_pj_ps,
                tc.tile_pool(name="a_kv", bufs=cfg["a_kv"]) as a_kv,
                tc.tile_pool(name="a_tmp", bufs=3) as a_tmp,
            ):
                def project(xblk, WT, brow, j, name):
                    """[s=128, o=512] psum halves of a projection.

                    xblk free layout is (c, s): col = c*SBLK + s.
                    """
                    halves = []
                    for h in range(2):
                        pj = a_pj_ps.tile([P, SBLK], f32, name="pj" if cfg.get("a_shared") else f"{name}_pj")
                        for c in range(EC):
                            nc.tensor.matmul(
                                pj[:],
                                xblk[:, c * SBLK + j * P : c * SBLK + (j + 1) * P],
                                WT[c][:, h * SBLK : (h + 1) * SBLK],
                                start=(c == 0),
                                stop=(brow is None and c == EC - 1),
                            )
                        if brow is not None:
                            nc.tensor.matmul(
                                pj[:],
                                ones_row[:, 0:P],
                                brow[:, h * SBLK : (h + 1) * SBLK],
                                start=False,
                                stop=True,
                            )
                        halves.append(pj)
                    return halves

                def emit_scores(kv_sb, gj):
                    for hh in range(H):
                        nc.tensor.matmul(
                            scores_ps[:, hh * D : (hh + 1) * D],
                            kv_sb[:, 2 * D * hh : 2 * D * (hh + 1)],
                            kv_sb[:, 2 * D * hh + D : 2 * D * (hh + 1)],
                            start=False,
                            stop=(gj == ST - 1 and hh == H - 1),
                            skip_group_check=True,
                        )

                kv_fifo = []  # (kv_sb, global_subtile), lag cfg["sc_lag"]
                sc_lag = cfg.get("sc_lag", 1)
                xq0 = None
                for bi in range(NBLK):
                    xk_nxt = xv_nxt = None
                    if bi + 1 < NBLK:
                        xk_nxt = load_xblk(xkT, bi + 1, "xk_blk")
                        xv_nxt = load_xblk(xvT, bi + 1, "xv_blk")
                    if bi == NBLK - 1:
                        # prefetch the first q block so phase C starts hot
                        xq0 = load_xq(0)
                    for j in range(JB):
                        gj = bi * JB + j
                        # per-head interleave: head hh cols [128*hh, 128*hh+128)
                        # v in the low 64, elu(k) in the high 64
                        kv_sb = a_kv.tile([P, 2 * E], bf, name="kv_sb")
                        kv4 = kv_sb[:].rearrange(
                            "p (hh two) -> p hh two", two=2 * D
                        )
                        kp = project(xk_cur, WkT, bk_row, j, "k")
                        for h in range(2):
                            r_sb = a_tmp.tile([P, SBLK], bf, name="kr_sb")
                            t_sb = a_tmp.tile([P, SBLK], bf, name="kt_sb")
                            e_sb = a_tmp.tile([P, SBLK], bf, name="ke_sb")
                            # relu on DVE to balance the Act engine (Exp+copies)
                            nc.vector.tensor_scalar(
                                r_sb[:], kp[h][:], 0.0, None, ALU.max
                            )
                            # elu(x) = relu(x) + min(exp(x), 1) - 1
                            nc.scalar.activation(e_sb[:], kp[h][:], AF.Exp)
                            nc.vector.tensor_scalar(
                                t_sb[:], e_sb[:], 1.0, -1.0, ALU.min, ALU.add
                            )
                            nc.gpsimd.tensor_tensor(
                                kv4[:, 8 * h : 8 * (h + 1), D : 2 * D],
                                t_sb[:].rearrange("p (hh d) -> p hh d", d=D),
                                r_sb[:].rearrange("p (hh d) -> p hh d", d=D),
                                ALU.add,
                            )
                        vp = project(xv_cur, WvT, bv_row, j, "v")
                        for h in range(2):
                            nc.scalar.copy(
                                kv4[:, 8 * h : 8 * (h + 1), 0:D],
                                vp[h][:].rearrange("p (hh d) -> p hh d", d=D),
                            )
                        if dbg and gj == 0:
                            kvd = dbgpool.tile([P, 2 * E], f32, name="kv_dbg")
                            nc.vector.tensor_copy(kvd[:], kv_sb[:])
                            nc.sync.dma_start(dbg_kv, kvd[:])
                        # scores lag so the PE never waits on the ELU chain
                        kv_fifo.append((kv_sb, gj))
                        if len(kv_fifo) > sc_lag:
                            emit_scores(*kv_fifo.pop(0))
                    xk_cur, xv_cur = xk_nxt, xv_nxt
                for ent in kv_fifo:
                    emit_scores(*ent)

                # -- extract scoresT + ksumsq while phase-A psum still alive
                # Gram rows (64:128) hold k^T k per head; diagonal = ksumsq
                gram_sb = small.tile([D, H * D], f32, name="gram_sb")
                nc.scalar.copy(gram_sb[:], scores_ps[D:P, :])
                gram_dram = drpool.tile([1, D * H * D], f32, name="gram_dram")
                nc.sync.dma_start(
                    gram_dram[:].rearrange("1 (d c) -> d c", d=D), gram_sb[:]
                )
                # diag idx for (hh, d) = d*(H*D) + hh*D + d = d*(H*D+1) + D*hh
                kcol = small.tile([P, EC], f32, name="kcol")
                gd = gram_dram[:].tensor
                for h2 in range(2):
                    src_ap = bass.AP(gd, h2 * D, [[H * D + 1, D], [2 * D, EC]])
                    nc.sync.dma_start(kcol[h2 * D : (h2 + 1) * D, :], src_ap)
                knorm = small.tile([P, EC], f32, name="knorm")
                nc.scalar.activation(knorm[:], kcol[:], AF.Sqrt, scale=float(S))
                invk = small.tile([P, EC], f32, name="invk")
                nc.vector.reciprocal(invk[:], knorm[:])
                if dbg:
                    nc.sync.dma_start(dbg_gram, gram_sb[:])
                    nc.sync.dma_start(dbg_invk, invk[:])

                bd = []
                for pr in range(EC):
                    h0, h1 = 2 * pr, 2 * pr + 1
                    nc.vector.tensor_copy(
                        bd_st[pr][0:D, 0:D], scores_ps[0:D, h0 * D : (h0 + 1) * D]
                    )
                    odd_stage = small.tile([D, D], f32, name="odd_stage")
                    nc.vector.tensor_copy(
                        odd_stage[:], scores_ps[0:D, h1 * D : (h1 + 1) * D]
                    )
                    nc.sync.dma_start(bd_st[pr][D:P, D:P], odd_stage[:])
                    bd_t = small.tile([P, P], bf, name=f"bd_{pr}")
                    nc.gpsimd.tensor_copy(bd_t[:], bd_st[pr][:])
                    bd.append(bd_t)

            a_sc_scope.__exit__(None, None, None)
            a_in_scope.__exit__(None, None, None)

            # ================= PHASES B + C, software-pipelined ===========
            # Emission order: C-pre(0) | B (W2 build) | C-pre(1) C-post(0) |
            # C-pre(2) C-post(1) | ... — the W2 build and each block's
            # inv-norm chain overlap the next block's projection matmuls, so
            # the PE never waits on the DVE/Act epilogues.
            w2scope = tc.tile_pool(name="w2pool", bufs=1)
            w2pool = w2scope.__enter__()
            W2 = [w2pool.tile([P, E], bf, name=f"W2_{c}") for c in range(EC)]
            with (
                tc.tile_pool(name="c_qt", bufs=4) as c_qt,
                tc.tile_pool(name="c_nrm", bufs=4) as c_nrm,
                tc.tile_pool(name="c_qn", bufs=1) as c_qn,
                tc.tile_pool(name="c_q2", bufs=1) as c_q2,
                tc.tile_pool(name="c_tmp", bufs=3) as c_tmp,
                tc.tile_pool(name="c_out", bufs=2) as c_out,
                # PSUM is a stack allocator: pools created first take the
                # lowest banks, which phase A's scores pool just vacated and
                # which only free after the Gram/bd epilogue reads.  Put the
                # late-needed fin/qb pools there; q_pj (needed immediately)
                # then lands on the early-freed projection banks.
                tc.tile_pool(name="c_fin_ps", bufs=cfg["c_fin"], space="PSUM") as c_fin_ps,
                tc.tile_pool(name="c_qb_ps", bufs=1, space="PSUM") as c_qb_ps,
                tc.tile_pool(name="c_ss_ps", bufs=2, space="PSUM") as c_ss_ps,
                tc.tile_pool(name="c_pj_ps", bufs=cfg["c_pj"], space="PSUM") as c_pj_ps,
            ):
                def c_pre(bi, xq_cur):
                    """q projection + ELU + compact row-norm for block bi.

                    The norm uses the [H, SBLK] compact form so the DVE
                    reciprocal (~6 HW cycles/element) runs once per block.
                    """
                    qt_tiles = []
                    q2_tiles = []
                    for ot in range(EC):
                        pj = c_pj_ps.tile([P, SBLK], f32, name="q_pj")
                        for c in range(EC):
                            nc.tensor.matmul(
                                pj[:],
                                WqT[c][:, ot * P : (ot + 1) * P],
                                xq_cur[:, c * SBLK : (c + 1) * SBLK],
                                start=(c == 0),
                                stop=(bq_row is None and c == EC - 1),
                            )
                        if bq_row is not None:
                            nc.tensor.matmul(
                                pj[:],
                                bq_row[:, ot * P : (ot + 1) * P],
                                ones_row[:],
                                start=False,
                                stop=True,
                            )
                        r_sb = c_tmp.tile([P, SBLK], bf, name="qr_sb")
                        t_sb = c_tmp.tile([P, SBLK], bf, name="qt_sb")
                        e_sb = c_tmp.tile([P, SBLK], bf, name="qe_sb")
                        qt_ = c_qt.tile([P, SBLK], bf, name=f"qt_{ot}")
                        nc.scalar.activation(r_sb[:], pj[:], AF.Relu)
                        # elu(x) = relu(x) + min(exp(x), 1) - 1
                        nc.scalar.activation(e_sb[:], pj[:], AF.Exp)
                        nc.vector.tensor_scalar(
                            t_sb[:], e_sb[:], 1.0, -1.0, ALU.min, ALU.add
                        )
                        nc.vector.tensor_tensor(qt_[:], t_sb[:], r_sb[:], ALU.add)
                        qt_tiles.append(qt_)
                        q2 = c_q2.tile([P, SBLK], bf, name=f"q2_{ot}")
                        # SBUF-only square on the idle GPSIMD engine
                        nc.gpsimd.tensor_tensor(q2[:], qt_[:], qt_[:], ALU.mult)
                        q2_tiles.append(q2)
                    qss_ps = c_ss_ps.tile([H, SBLK], f32, name="qss_ps")
                    for ot in range(EC):
                        nc.tensor.matmul(
                            qss_ps[:],
                            blockones[ot][:],
                            q2_tiles[ot][:],
                            start=(ot == 0),
                            stop=(ot == EC - 1),
                        )
                    # invq = 1 / sqrt(D * qss), emitted bf16 for PE broadcast
                    # (Sqrt first, on moderate-magnitude inputs: the Act-engine
                    # Sqrt table is inaccurate for tiny inputs)
                    qss_sb = c_nrm.tile([H, SBLK], f32, name="qss_sb")
                    nc.scalar.activation(
                        qss_sb[:], qss_ps[:], AF.Sqrt, scale=float(D)
                    )
                    invq = c_nrm.tile([H, SBLK], f32, name="invq")
                    nc.vector.reciprocal(invq[:], qss_sb[:])
                    invq_r = c_nrm.tile([H, SBLK], bf, name="invq_r")
                    nc.vector.tensor_copy(invq_r[:], invq[:])
                    if dbg and bi == 0:
                        for ot in range(EC):
                            qtd = dbgpool.tile([P, SBLK], f32, name="qt_dbg")
                            nc.vector.tensor_copy(qtd[:], qt_tiles[ot][:])
                            nc.sync.dma_start(
                                dbg_qt[ot * P : (ot + 1) * P, :], qtd[:]
                            )
                    return (qt_tiles, invq_r)

                def c_post(bi, qt_tiles, invq_r):
                    """inv-norm broadcast, q scaling, fused output GEMM."""
                    s0 = bi * SBLK
                    qn_tiles = []
                    for ot in range(EC):
                        qb = c_qb_ps.tile([P, SBLK], f32, name="qb_ps")
                        nc.tensor.matmul(
                            qb[:], blockpick[ot], invq_r[:],
                            start=True, stop=True,
                        )
                        qn = c_qn.tile([P, SBLK], bf, name=f"qn_{ot}")
                        nc.vector.tensor_tensor(
                            qn[:], qt_tiles[ot][:], qb[:], ALU.mult
                        )
                        if dbg and bi == 0:
                            qnd = dbgpool.tile([P, SBLK], f32, name="qn_dbg")
                            nc.vector.tensor_copy(qnd[:], qn[:])
                            nc.sync.dma_start(
                                dbg_qn[ot * P : (ot + 1) * P, :], qnd[:]
                            )
                        qn_tiles.append(qn)
                    for j2 in range(JB // 2):
                        o_sb = c_out.tile([P, 2 * E], f32, name="o_sb")
                        for tj in range(2):
                            j = j2 * 2 + tj
                            for h in range(2):
                                fin = c_fin_ps.tile([P, SBLK], f32, name="fin_ps")
                                for c in range(EC):
                                    nc.tensor.matmul(
                                        fin[:],
                                        qn_tiles[c][:, j * P : (j + 1) * P],
                                        W2[c][:, h * SBLK : (h + 1) * SBLK],
                                        start=(c == 0),
                                        stop=(bo_row is None and c == EC - 1),
                                    )
                                if bo_row is not None:
                                    nc.tensor.matmul(
                                        fin[:],
                                        ones_row[:, 0:P],
                                        bo_row[:, h * SBLK : (h + 1) * SBLK],
                                        start=False,
                                        stop=True,
                                    )
                                sl = slice(tj * E + h * SBLK, tj * E + (h + 1) * SBLK)
                                osb_mode = cfg.get("osb", "alt")
                                if osb_mode == "act" or (
                                    osb_mode == "alt" and (j + h) % 2 == 1
                                ):
                                    nc.scalar.copy(o_sb[:, sl], fin[:])
                                else:
                                    nc.vector.tensor_copy(o_sb[:, sl], fin[:])
                            # store per 128-row tile so the final store isn't
                            # serialized behind both tiles' copies
                            nc.sync.dma_start(
                                out[s0 + j * P : s0 + (j + 1) * P, :],
                                o_sb[:, tj * E : (tj + 1) * E],
                            )

                def emit_w2():
                    """W2 = knorm^-1 * scoresT @ WoT; psums share the fin ring."""
                    for c in range(EC):
                        for h in range(2):
                            w2p = c_fin_ps.tile([P, SBLK], f32, name="fin_ps")
                            nc.tensor.matmul(
                                w2p[:],
                                bd[c][:],
                                WoT[c][:, h * SBLK : (h + 1) * SBLK],
                                start=True,
                                stop=True,
                            )
                            dst = W2[c][:, h * SBLK : (h + 1) * SBLK]
                            if (c + h) % 2 == 0:
                                nc.vector.tensor_scalar(
                                    dst, w2p[:], invk[:, c : c + 1], None, ALU.mult
                                )
                            else:
                                nc.scalar.activation(
                                    dst, w2p[:], AF.Copy, scale=invk[:, c : c + 1]
                                )
                    if dbg:
                        for c in range(EC):
                            w2d = dbgpool.tile([P, E], f32, name="w2_dbg")
                            nc.vector.tensor_copy(w2d[:], W2[c][:])
                            nc.sync.dma_start(dbg_w2[c * P : (c + 1) * P, :], w2d[:])

                # lag-2 software pipeline: posts trail pres by two blocks so
                # the W2 build and each block's inv-norm chain are covered by
                # ~30us of independent PE work
                assert NBLK >= 3
                xq_cur = xq0  # prefetched during phase A
                xq_nxt = load_xq(1)
                pres = [c_pre(0, xq_cur)]
                xq_cur, xq_nxt = xq_nxt, load_xq(2)
                pres.append(c_pre(1, xq_cur))
                emit_w2()
                for bi in range(2, NBLK):
                    xq_cur = xq_nxt
                    xq_nxt = load_xq(bi + 1) if bi + 1 < NBLK else None
                    pres.append(c_pre(bi, xq_cur))
                    c_post(bi - 2, *pres[bi - 2])
                    pres[bi - 2] = None
                c_post(NBLK - 2, *pres[NBLK - 2])
                c_post(NBLK - 1, *pres[NBLK - 1])
            w2scope.__exit__(None, None, None)
            c_in_scope.__exit__(None, None, None)

    _patch_bass(nc)
    return nc


# --------------------------------------------------------------------------
# Host wrapper
# --------------------------------------------------------------------------
_NC_CACHE = {}


def _get_nc(S, with_bias=False):
    key = (S, with_bias)
    if key not in _NC_CACHE:
        _NC_CACHE[key] = build(S, with_bias)
    return _NC_CACHE[key]


def _t_bf16(x):
    """[S, E] f32 -> [E, S] bf16 contiguous."""
    return np.ascontiguousarray(np.asarray(x, np.float32).astype(BF16).T)


def _bpick_const():
    """[H, EC*P]: slice ot is blockones_ot^T (per-head broadcast mask)."""
    bp = np.zeros((H, EC * P), np.float32)
    for ot in range(EC):
        bp[2 * ot, ot * P : ot * P + D] = 1.0
        bp[2 * ot + 1, ot * P + D : (ot + 1) * P] = 1.0
    return np.ascontiguousarray(bp.astype(BF16))


def make_in_maps(query, key, value, Wq, bq, Wk, bk, Wv, bv, Wo, bo):
    query = np.asarray(query, np.float32)
    B = query.shape[0]
    with_bias = any(np.any(np.asarray(b)) for b in (bq, bk, bv, bo))
    shared = {
        "WqT": _t_bf16(Wq),
        "WkT": _t_bf16(Wk),
        "WvT": _t_bf16(Wv),
        "WoT": _t_bf16(Wo),
        "bpick": _bpick_const(),
    }
    if with_bias:
        for name, b in (("bqr", bq), ("bkr", bk), ("bvr", bv), ("bor", bo)):
            shared[name] = np.ascontiguousarray(
                np.asarray(b, np.float32).reshape(1, E).astype(BF16)
            )
    return [
        {
            "xqT": _t_bf16(query[c]),
            "xkT": _t_bf16(key[c]),
            "xvT": _t_bf16(value[c]),
            **shared,
        }
        for c in range(B)
    ]


def kernel(query, key, value, Wq, bq, Wk, bk, Wv, bv, Wo, bo):
    query = np.asarray(query, np.float32)
    B, S, E_ = query.shape
    assert E_ == E and B == N_CORES
    in_maps = make_in_maps(query, key, value, Wq, bq, Wk, bk, Wv, bv, Wo, bo)
    with_bias = any(np.any(np.asarray(b)) for b in (bq, bk, bv, bo))
    nc = _get_nc(S, with_bias)
    res = run_bass_kernel_spmd(nc, in_maps, core_ids=list(range(N_CORES)))
    return np.stack([res.results[c]["out"] for c in range(B)])


# revision 75
# speedup vs baseline: 1.0046x; 1.0046x over previous
"""Trainium2 Bass kernel for nn_MultiHeadedLinrec (linear attention).

Math (per batch element, reference semantics):
    q = elu(x_q @ Wq.T + bq)    [S, E] viewed as [S, H, d]
    k = elu(x_k @ Wk.T + bk)
    v = x_v @ Wv.T + bv
    k <- k / (||k||_seq * sqrt(S))     (per (h, d) column norm over S)
    q <- q / (||q||_d   * sqrt(d))     (per (s, h) row norm over d)
    scores_h = k_h^T @ v_h             [d, d]
    out = concat_h(q_h @ scores_h) @ Wo.T + bo

Kernel strategy (one NeuronCore per batch element, 8 cores data-parallel):
  Host pre-transposes x_q/x_k/x_v to [E, S] and converts x + weights to
  bf16, so the device never runs PE transposes and every matmul streams
  bf16 moving operands at 1 cyc/row (fp32 PSUM accumulation throughout).

  Phase A (stream S in 512-col blocks, 128-row subtiles): project k and v
    into natural [s, e] layout (data chunks stationary, weight chunks
    moving), ELU(k), pack per-head [v|k] bf16 tiles, accumulate per-head
    scoresT = v_h^T k_h and the k Gram matrix on the PE (scores matmuls are
    emitted with a one-subtile lag so the in-order PE queue never waits on
    the ELU chain).
  Phase B: knorm from the Gram diagonal (DMA diag gather), then fold
    k-norm + scoresT + Wo into one fused weight
    W2[i, o] = (scores @ Wo.T)[i, o] / (knorm[i] * sqrt(S)), bf16.
  Phase C (stream S in 512-col blocks): q projection in transposed [e, s]
    layout (weight chunks stationary, data moving), ELU, per-head row
    norms via block-ones matmul + PE broadcast, then out = qn^T @ W2 + bo
    in natural layout.  B and C are software-pipelined: the W2 build and
    each block's inv-norm chain trail the projection stream by two blocks,
    so the in-order PE queue never waits on DVE/Act epilogues.  PSUM pools
    are created so late-needed pools take the banks phase A frees last.

This walrus build only supports ONE sync wait per instruction; Tile emits
multi-wait instructions, so we legalize the BIR JSON by hoisting extra waits
onto inserted NoOps (see _legalize_sync_json).
"""

import json

import ml_dtypes
import numpy as np

import concourse.bass as bass
import concourse.mybir as mybir
import concourse.tile as tile
from concourse.bass_utils import run_bass_kernel_spmd

dt = mybir.dt
AF = mybir.ActivationFunctionType
ALU = mybir.AluOpType

P = 128
E = 1024
H = 16
D = 64
N_CORES = 8
EC = E // P  # 8 chunks of 128 along the embedding dim
SBLK = 512  # s-block width
BF16 = ml_dtypes.bfloat16


# --------------------------------------------------------------------------
# BIR sync legalization: max one wait / one update per instruction.
# --------------------------------------------------------------------------
def _legalize_sync_json(bir_json: bytes) -> bytes:
    m = json.loads(bir_json)
    counter = [0]

    def fresh():
        counter[0] += 1
        return f"I-synclift-{counter[0]}"

    for f in m["functions"]:
        for blk in f["blocks"]:
            out = []
            for ins in blk["instructions"]:
                si = ins.get("sync_info")
                if not si:
                    out.append(ins)
                    continue
                waits = si.get("on_wait") or []
                updates = si.get("on_update") or []
                if len(waits) <= 1 and len(updates) <= 1:
                    out.append(ins)
                    continue
                eng = ins.get("engine")
                dbg = ins.get("debug")
                for w in waits[:-1]:
                    out.append(
                        {
                            "debug": dbg,
                            "engine": eng,
                            "ins": [],
                            "name": fresh(),
                            "opcode": "NoOp",
                            "outs": [],
                            "sync_info": {"on_update": [], "on_wait": [w]},
                        }
                    )
                si["on_wait"] = waits[-1:]
                post = [
                    {
                        "debug": dbg,
                        "engine": eng,
                        "ins": [],
                        "name": fresh(),
                        "opcode": "NoOp",
                        "outs": [],
                        "sync_info": {"on_update": [u], "on_wait": []},
                    }
                    for u in updates[1:]
                ]
                si["on_update"] = updates[:1]
                out.append(ins)
                out.extend(post)
            blk["instructions"] = out
    return json.dumps(m).encode()


def _patch_bass(nc):
    orig = nc.to_json_bytes

    def patched():
        return _legalize_sync_json(orig())

    nc.to_json_bytes = patched
    return nc


# --------------------------------------------------------------------------
# Kernel builder
# --------------------------------------------------------------------------
def build(S: int = 4096, with_bias: bool = False, cfg: dict | None = None):
    cfg = {"a_pj": 3, "a_kv": 3, "c_pj": 2, "c_fin": 3, "sc_lag": 2, **(cfg or {})}
    dbg = cfg.get("debug", False)
    ST = S // P  # number of 128-row s-tiles
    NBLK = S // SBLK
    JB = SBLK // P  # s-tiles per block (4)

    nc = bass.Bass(trn_type="TRN2", target_bir_lowering=False, debug=False)

    bf = dt.bfloat16
    f32 = dt.float32

    xqT = nc.dram_tensor("xqT", [E, S], bf, kind="ExternalInput").ap()
    xkT = nc.dram_tensor("xkT", [E, S], bf, kind="ExternalInput").ap()
    xvT = nc.dram_tensor("xvT", [E, S], bf, kind="ExternalInput").ap()
    WqTd = nc.dram_tensor("WqT", [E, E], bf, kind="ExternalInput").ap()
    WkTd = nc.dram_tensor("WkT", [E, E], bf, kind="ExternalInput").ap()
    WvTd = nc.dram_tensor("WvT", [E, E], bf, kind="ExternalInput").ap()
    WoTd = nc.dram_tensor("WoT", [E, E], bf, kind="ExternalInput").ap()
    if with_bias:
        bqr = nc.dram_tensor("bqr", [1, E], bf, kind="ExternalInput").ap()
        bkr = nc.dram_tensor("bkr", [1, E], bf, kind="ExternalInput").ap()
        bvr = nc.dram_tensor("bvr", [1, E], bf, kind="ExternalInput").ap()
        bor = nc.dram_tensor("bor", [1, E], bf, kind="ExternalInput").ap()
    bpickd = nc.dram_tensor("bpick", [H, EC * P], bf, kind="ExternalInput").ap()
    out = nc.dram_tensor("out", [S, E], f32, kind="ExternalOutput").ap()
    if dbg:
        dbg_kv = nc.dram_tensor("dbg_kv", [P, 2 * E], f32, kind="ExternalOutput").ap()
        dbg_gram = nc.dram_tensor("dbg_gram", [D, H * D], f32, kind="ExternalOutput").ap()
        dbg_invk = nc.dram_tensor("dbg_invk", [P, EC], f32, kind="ExternalOutput").ap()
        dbg_w2 = nc.dram_tensor("dbg_w2", [E, E], f32, kind="ExternalOutput").ap()
        dbg_qt = nc.dram_tensor("dbg_qt", [E, SBLK], f32, kind="ExternalOutput").ap()
        dbg_qn = nc.dram_tensor("dbg_qn", [E, SBLK], f32, kind="ExternalOutput").ap()

    with tile.TileContext(nc) as tc:
        with (
            tc.tile_pool(name="consts", bufs=1) as consts,
            tc.tile_pool(name="small", bufs=1) as small,
            tc.tile_pool(name="wts", bufs=1) as wts,
            tc.tile_pool(name="drpool", bufs=1, space="DRAM") as drpool,
            tc.tile_pool(name="dbgpool", bufs=1) as dbgpool,
        ):
            # ---------------- constants ----------------
            # zrow first: the scores-psum zeroing matmuls are the PE's first
            # instructions and should not wait on the other const memsets
            zrow = consts.tile([1, SBLK], bf, name="zrow")
            nc.vector.memset(zrow[:], 0.0)
            # blockones_ot [P, H]: col 2*ot -> partitions 0:64, col 2*ot+1 ->
            # partitions 64:128 (head-pair sum masks for qss accumulation)
            blockones = []
            for c in range(EC):
                t = consts.tile([P, H], bf, name=f"blockones_{c}")
                nc.vector.memset(t[:], 0.0)
                nc.vector.memset(t[0:D, 2 * c : 2 * c + 1], 1.0)
                nc.vector.memset(t[D:P, 2 * c + 1 : 2 * c + 2], 1.0)
                blockones.append(t)
            # blockpick_ot [H, P] = blockones_ot^T (broadcast masks); loaded
            # from a host constant because DVE memsets cannot start at odd
            # partitions.
            bpick_all = consts.tile([H, EC * P], bf, name="bpick_all")
            nc.sync.dma_start(bpick_all[:], bpickd)
            blockpick = [bpick_all[:, c * P : (c + 1) * P] for c in range(EC)]
            ones_row = None
            if with_bias:
                ones_row = consts.tile([1, SBLK], bf, name="ones_row")
                nc.vector.memset(ones_row[:], 1.0)

            # bd staging (scoresT block-diagonal per chunk), zeroed once
            bd_st = []
            for pr in range(EC):
                s_t = small.tile([P, P], f32, name=f"bd_st_{pr}")
                nc.vector.memset(s_t[:], 0.0)
                bd_st.append(s_t)

            # ---------------- weights (all bf16, straight DMA) -----------
            def load_wt(WTd, name):
                tiles = []
                for c in range(EC):
                    t = wts.tile([P, E], bf, name=f"{name}T_{c}")
                    nc.sync.dma_start(t[:], WTd[c * P : (c + 1) * P, :])
                    tiles.append(t)
                return tiles

            def load_brow(src, name):
                t = small.tile([1, E], bf, name=name)
                nc.sync.dma_start(t[:], src)
                return t

            bk_row = bv_row = bq_row = bo_row = None
            if with_bias:
                bk_row = load_brow(bkr, "bk_row")
                bv_row = load_brow(bvr, "bv_row")

            # ================= PHASE A ====================================
            c_in_scope = tc.tile_pool(name="c_in", bufs=2)
            c_in = c_in_scope.__enter__()
            a_in_scope = tc.tile_pool(name="a_in", bufs=2)
            a_in = a_in_scope.__enter__()

            def load_xq(bi):
                t = c_in.tile([P, EC * SBLK], bf, name="xq_blk")
                nc.sync.dma_start(
                    t[:].rearrange("p (c s) -> p c s", c=EC),
                    xqT[:, bi * SBLK : (bi + 1) * SBLK].rearrange(
                        "(c p) s -> p c s", p=P
                    ),
                )
                return t

            def load_xblk(src, bi, name):
                t = a_in.tile([P, EC * SBLK], bf, name=name)
                nc.sync.dma_start(
                    t[:].rearrange("p (c s) -> p c s", c=EC),
                    src[:, bi * SBLK : (bi + 1) * SBLK].rearrange(
                        "(c p) s -> p c s", p=P
                    ),
                )
                return t

            # Startup: interleave the k-weight and first-block loads per
            # chunk so the first projection matmuls start ~1us in (the DMA
            # device is serial; a monolithic 3MB prefix stalls the PE 12us)
            WkT = [wts.tile([P, E], bf, name=f"WkT_{c}") for c in range(EC)]
            WvT = [wts.tile([P, E], bf, name=f"WvT_{c}") for c in range(EC)]
            xk_cur = a_in.tile([P, EC * SBLK], bf, name="xk_blk")
            xv_cur = a_in.tile([P, EC * SBLK], bf, name="xv_blk")
            for c in range(EC):
                nc.sync.dma_start(WkT[c][:], WkTd[c * P : (c + 1) * P, :])
                nc.sync.dma_start(
                    xk_cur[:, c * SBLK : (c + 1) * SBLK],
                    xkT[c * P : (c + 1) * P, 0:SBLK],
                )
            for c in range(EC):
                nc.sync.dma_start(WvT[c][:], WvTd[c * P : (c + 1) * P, :])
                nc.sync.dma_start(
                    xv_cur[:, c * SBLK : (c + 1) * SBLK],
                    xvT[c * P : (c + 1) * P, 0:SBLK],
                )
            WqT = load_wt(WqTd, "Wq")
            WoT = load_wt(WoTd, "Wo")
            if with_bias:
                bq_row = load_brow(bqr, "bq_row")
                bo_row = load_brow(bor, "bo_row")

            a_sc_scope = tc.tile_pool(name="a_sc_ps", bufs=1, space="PSUM")
            a_sc_ps = a_sc_scope.__enter__()
            scores_ps = a_sc_ps.tile([P, H * D], f32, name="scores_ps")
            # Pre-zero via start=True matmuls: a matmul's start bit zeroes the
            # whole PSUM bank, so the per-head 64-col accumulation below must
            # never use start=True (it would erase sibling heads' partials).
            for hb in range(2):
                nc.tensor.matmul(
                    scores_ps[:, hb * SBLK : (hb + 1) * SBLK],
                    zrow[:, 0:P],
                    zrow[:],
                    start=True,
                    stop=True,
                    skip_group_check=True,
                )

            with (
                tc.tile_pool(name="a_pj_ps", bufs=cfg["a_pj"] * (2 if cfg.get("a_shared") else 1), space="PSUM") as a_pj_ps,
                tc.tile_pool(name="a_kv", bufs=cfg["a_kv"]) as a_kv,
                tc.tile_pool(name="a_tmp", bufs=3) as a_tmp,
            ):
                def project(xblk, WT, brow, j, name):
                    """[s=128, o=512] psum halves of a projection.

                    xblk free layout is (c, s): col = c*SBLK + s.
                    """
                    halves = []
                    for h in range(2):
                        pj = a_pj_ps.tile([P, SBLK], f32, name="pj" if cfg.get("a_shared") else f"{name}_pj")
                        for c in range(EC):
                            nc.tensor.matmul(
                                pj[:],
                                xblk[:, c * SBLK + j * P : c * SBLK + (j + 1) * P],
                                WT[c][:, h * SBLK : (h + 1) * SBLK],
                                start=(c == 0),
                                stop=(brow is None and c == EC - 1),
                            )
                        if brow is not None:
                            nc.tensor.matmul(
                                pj[:],
                                ones_row[:, 0:P],
                                brow[:, h * SBLK : (h + 1) * SBLK],
                                start=False,
                                stop=True,
                            )
                        halves.append(pj)
                    return halves

                def emit_scores(kv_sb, gj):
                    for hh in range(H):
                        nc.tensor.matmul(
                            scores_ps[:, hh * D : (hh + 1) * D],
                            kv_sb[:, 2 * D * hh : 2 * D * (hh + 1)],
                            kv_sb[:, 2 * D * hh + D : 2 * D * (hh + 1)],
                            start=False,
                            stop=(gj == ST - 1 and hh == H - 1),
                            skip_group_check=True,
                        )

                kv_fifo = []  # (kv_sb, global_subtile), lag cfg["sc_lag"]
                sc_lag = cfg.get("sc_lag", 1)
                xq0 = None
                for bi in range(NBLK):
                    xk_nxt = xv_nxt = None
                    if bi + 1 < NBLK:
                        xk_nxt = load_xblk(xkT, bi + 1, "xk_blk")
                        xv_nxt = load_xblk(xvT, bi + 1, "xv_blk")
                    if bi == NBLK - 1:
                        # prefetch the first q block so phase C starts hot
                        xq0 = load_xq(0)
                    for j in range(JB):
                        gj = bi * JB + j
                        # per-head interleave: head hh cols [128*hh, 128*hh+128)
                        # v in the low 64, elu(k) in the high 64
                        kv_sb = a_kv.tile([P, 2 * E], bf, name="kv_sb")
                        kv4 = kv_sb[:].rearrange(
                            "p (hh two) -> p hh two", two=2 * D
                        )
                        kp = project(xk_cur, WkT, bk_row, j, "k")
                        for h in range(2):
                            r_sb = a_tmp.tile([P, SBLK], bf, name="kr_sb")
                            t_sb = a_tmp.tile([P, SBLK], bf, name="kt_sb")
                            e_sb = a_tmp.tile([P, SBLK], bf, name="ke_sb")
                            # relu on DVE to balance the Act engine (Exp+copies)
                            nc.vector.tensor_scalar(
                                r_sb[:], kp[h][:], 0.0, None, ALU.max
                            )
                            # elu(x) = relu(x) + min(exp(x), 1) - 1
                            nc.scalar.activation(e_sb[:], kp[h][:], AF.Exp)
                            nc.vector.tensor_scalar(
                                t_sb[:], e_sb[:], 1.0, -1.0, ALU.min, ALU.add
                            )
                            nc.gpsimd.tensor_tensor(
                                kv4[:, 8 * h : 8 * (h + 1), D : 2 * D],
                                t_sb[:].rearrange("p (hh d) -> p hh d", d=D),
                                r_sb[:].rearrange("p (hh d) -> p hh d", d=D),
                                ALU.add,
                            )
                        vp = project(xv_cur, WvT, bv_row, j, "v")
                        for h in range(2):
                            nc.scalar.copy(
                                kv4[:, 8 * h : 8 * (h + 1), 0:D],
                                vp[h][:].rearrange("p (hh d) -> p hh d", d=D),
                            )
                        if dbg and gj == 0:
                            kvd = dbgpool.tile([P, 2 * E], f32, name="kv_dbg")
                            nc.vector.tensor_copy(kvd[:], kv_sb[:])
                            nc.sync.dma_start(dbg_kv, kvd[:])
                        # scores lag so the PE never waits on the ELU chain
                        kv_fifo.append((kv_sb, gj))
                        if len(kv_fifo) > sc_lag:
                            emit_scores(*kv_fifo.pop(0))
                    xk_cur, xv_cur = xk_nxt, xv_nxt
                for ent in kv_fifo:
                    emit_scores(*ent)

                # -- extract scoresT + ksumsq while phase-A psum still alive
                # Gram rows (64:128) hold k^T k per head; diagonal = ksumsq
                gram_sb = small.tile([D, H * D], f32, name="gram_sb")
                nc.scalar.copy(gram_sb[:], scores_ps[D:P, :])
                gram_dram = drpool.tile([1, D * H * D], f32, name="gram_dram")
                nc.sync.dma_start(
                    gram_dram[:].rearrange("1 (d c) -> d c", d=D), gram_sb[:]
                )
                # diag idx for (hh, d) = d*(H*D) + hh*D + d = d*(H*D+1) + D*hh
                kcol = small.tile([P, EC], f32, name="kcol")
                gd = gram_dram[:].tensor
                for h2 in range(2):
                    src_ap = bass.AP(gd, h2 * D, [[H * D + 1, D], [2 * D, EC]])
                    nc.sync.dma_start(kcol[h2 * D : (h2 + 1) * D, :], src_ap)
                knorm = small.tile([P, EC], f32, name="knorm")
                nc.scalar.activation(knorm[:], kcol[:], AF.Sqrt, scale=float(S))
                invk = small.tile([P, EC], f32, name="invk")
                nc.vector.reciprocal(invk[:], knorm[:])
                if dbg:
                    nc.sync.dma_start(dbg_gram, gram_sb[:])
                    nc.sync.dma_start(dbg_invk, invk[:])

                bd = []
                for pr in range(EC):
                    h0, h1 = 2 * pr, 2 * pr + 1
                    nc.vector.tensor_copy(
                        bd_st[pr][0:D, 0:D], scores_ps[0:D, h0 * D : (h0 + 1) * D]
                    )
                    odd_stage = small.tile([D, D], f32, name="odd_stage")
                    nc.vector.tensor_copy(
                        odd_stage[:], scores_ps[0:D, h1 * D : (h1 + 1) * D]
                    )
                    nc.sync.dma_start(bd_st[pr][D:P, D:P], odd_stage[:])
                    bd_t = small.tile([P, P], bf, name=f"bd_{pr}")
                    nc.gpsimd.tensor_copy(bd_t[:], bd_st[pr][:])
                    bd.append(bd_t)

            a_sc_scope.__exit__(None, None, None)
            a_in_scope.__exit__(None, None, None)

            # ================= PHASES B + C, software-pipelined ===========
            # Emission order: C-pre(0) | B (W2 build) | C-pre(1) C-post(0) |
            # C-pre(2) C-post(1) | ... — the W2 build and each block's
            # inv-norm chain overlap the next block's projection matmuls, so
            # the PE never waits on the DVE/Act epilogues.
            w2scope = tc.tile_pool(name="w2pool", bufs=1)
            w2pool = w2scope.__enter__()
            W2 = [w2pool.tile([P, E], bf, name=f"W2_{c}") for c in range(EC)]
            with (
                tc.tile_pool(name="c_qt", bufs=4) as c_qt,
                tc.tile_pool(name="c_nrm", bufs=4) as c_nrm,
                tc.tile_pool(name="c_qn", bufs=1) as c_qn,
                tc.tile_pool(name="c_q2", bufs=1) as c_q2,
                tc.tile_pool(name="c_tmp", bufs=3) as c_tmp,
                tc.tile_pool(name="c_out", bufs=2) as c_out,
                # PSUM is a stack allocator: pools created first take the
                # lowest banks, which phase A's scores pool just vacated and
                # which only free after the Gram/bd epilogue reads.  Put the
                # late-needed fin/qb pools there; q_pj (needed immediately)
                # then lands on the early-freed projection banks.
                tc.tile_pool(name="c_fin_ps", bufs=cfg["c_fin"], space="PSUM") as c_fin_ps,
                tc.tile_pool(name="c_qb_ps", bufs=1, space="PSUM") as c_qb_ps,
                tc.tile_pool(name="c_ss_ps", bufs=2, space="PSUM") as c_ss_ps,
                tc.tile_pool(name="c_pj_ps", bufs=cfg["c_pj"], space="PSUM") as c_pj_ps,
            ):
                def c_pre(bi, xq_cur):
                    """q projection + ELU + compact row-norm for block bi.

                    The norm uses the [H, SBLK] compact form so the DVE
                    reciprocal (~6 HW cycles/element) runs once per block.
                    """
                    qt_tiles = []
                    q2_tiles = []
                    for ot in range(EC):
                        pj = c_pj_ps.tile([P, SBLK], f32, name="q_pj")
                        for c in range(EC):
                            nc.tensor.matmul(
                                pj[:],
                                WqT[c][:, ot * P : (ot + 1) * P],
                                xq_cur[:, c * SBLK : (c + 1) * SBLK],
                                start=(c == 0),
                                stop=(bq_row is None and c == EC - 1),
                            )
                        if bq_row is not None:
                            nc.tensor.matmul(
                                pj[:],
                                bq_row[:, ot * P : (ot + 1) * P],
                                ones_row[:],
                                start=False,
                                stop=True,
                            )
                        r_sb = c_tmp.tile([P, SBLK], bf, name="qr_sb")
                        t_sb = c_tmp.tile([P, SBLK], bf, name="qt_sb")
                        e_sb = c_tmp.tile([P, SBLK], bf, name="qe_sb")
                        qt_ = c_qt.tile([P, SBLK], bf, name=f"qt_{ot}")
                        nc.scalar.activation(r_sb[:], pj[:], AF.Relu)
                        # elu(x) = relu(x) + min(exp(x), 1) - 1
                        nc.scalar.activation(e_sb[:], pj[:], AF.Exp)
                        nc.vector.tensor_scalar(
                            t_sb[:], e_sb[:], 1.0, -1.0, ALU.min, ALU.add
                        )
                        nc.vector.tensor_tensor(qt_[:], t_sb[:], r_sb[:], ALU.add)
                        qt_tiles.append(qt_)
                        q2 = c_q2.tile([P, SBLK], bf, name=f"q2_{ot}")
                        # SBUF-only square on the idle GPSIMD engine
                        nc.gpsimd.tensor_tensor(q2[:], qt_[:], qt_[:], ALU.mult)
                        q2_tiles.append(q2)
                    qss_ps = c_ss_ps.tile([H, SBLK], f32, name="qss_ps")
                    for ot in range(EC):
                        nc.tensor.matmul(
                            qss_ps[:],
                            blockones[ot][:],
                            q2_tiles[ot][:],
                            start=(ot == 0),
                            stop=(ot == EC - 1),
                        )
                    # invq = 1 / sqrt(D * qss), emitted bf16 for PE broadcast
                    # (Sqrt first, on moderate-magnitude inputs: the Act-engine
                    # Sqrt table is inaccurate for tiny inputs)
                    qss_sb = c_nrm.tile([H, SBLK], f32, name="qss_sb")
                    nc.scalar.activation(
                        qss_sb[:], qss_ps[:], AF.Sqrt, scale=float(D)
                    )
                    invq = c_nrm.tile([H, SBLK], f32, name="invq")
                    nc.vector.reciprocal(invq[:], qss_sb[:])
                    invq_r = c_nrm.tile([H, SBLK], bf, name="invq_r")
                    nc.vector.tensor_copy(invq_r[:], invq[:])
                    if dbg and bi == 0:
                        for ot in range(EC):
                            qtd = dbgpool.tile([P, SBLK], f32, name="qt_dbg")
                            nc.vector.tensor_copy(qtd[:], qt_tiles[ot][:])
                            nc.sync.dma_start(
                                dbg_qt[ot * P : (ot + 1) * P, :], qtd[:]
                            )
                    return (qt_tiles, invq_r)

                def c_post(bi, qt_tiles, invq_r):
                    """inv-norm broadcast, q scaling, fused output GEMM."""
                    s0 = bi * SBLK
                    qn_tiles = []
                    for ot in range(EC):
                        qb = c_qb_ps.tile([P, SBLK], f32, name="qb_ps")
                        nc.tensor.matmul(
                            qb[:], blockpick[ot], invq_r[:],
                            start=True, stop=True,
                        )
                        qn = c_qn.tile([P, SBLK], bf, name=f"qn_{ot}")
                        nc.vector.tensor_tensor(
                            qn[:], qt_tiles[ot][:], qb[:], ALU.mult
                        )
                        if dbg and bi == 0:
                            qnd = dbgpool.tile([P, SBLK], f32, name="qn_dbg")
                            nc.vector.tensor_copy(qnd[:], qn[:])
                            nc.sync.dma_start(
                                dbg_qn[ot * P : (ot + 1) * P, :], qnd[:]
                            )
                        qn_tiles.append(qn)
                    for j2 in range(JB // 2):
                        o_sb = c_out.tile([P, 2 * E], f32, name="o_sb")
                        for tj in range(2):
                            j = j2 * 2 + tj
                            for h in range(2):
                                fin = c_fin_ps.tile([P, SBLK], f32, name="fin_ps")
                                for c in range(EC):
                                    nc.tensor.matmul(
                                        fin[:],
                                        qn_tiles[c][:, j * P : (j + 1) * P],
                                        W2[c][:, h * SBLK : (h + 1) * SBLK],
                                        start=(c == 0),
                                        stop=(bo_row is None and c == EC - 1),
                                    )
                                if bo_row is not None:
                                    nc.tensor.matmul(
                                        fin[:],
                                        ones_row[:, 0:P],
                                        bo_row[:, h * SBLK : (h + 1) * SBLK],
                                        start=False,
                                        stop=True,
                                    )
                                sl = slice(tj * E + h * SBLK, tj * E + (h + 1) * SBLK)
                                osb_mode = cfg.get("osb", "alt")
                                if osb_mode == "act" or (
                                    osb_mode == "alt" and (j + h) % 2 == 1
                                ):
                                    nc.scalar.copy(o_sb[:, sl], fin[:])
                                else:
                                    nc.vector.tensor_copy(o_sb[:, sl], fin[:])
                            # store per 128-row tile so the final store isn't
                            # serialized behind both tiles' copies
                            nc.sync.dma_start(
                                out[s0 + j * P : s0 + (j + 1) * P, :],
                                o_sb[:, tj * E : (tj + 1) * E],
                            )

                def emit_w2():
                    """W2 = knorm^-1 * scoresT @ WoT; psums share the fin ring."""
                    for c in range(EC):
                        for h in range(2):
                            w2p = c_fin_ps.tile([P, SBLK], f32, name="fin_ps")
                            nc.tensor.matmul(
                                w2p[:],
                                bd[c][:],
                                WoT[c][:, h * SBLK : (h + 1) * SBLK],
                                start=True,
                                stop=True,
                            )
                            dst = W2[c][:, h * SBLK : (h + 1) * SBLK]
                            if (c + h) % 2 == 0:
                                nc.vector.tensor_scalar(
                                    dst, w2p[:], invk[:, c : c + 1], None, ALU.mult
                                )
                            else:
                                nc.scalar.activation(
                                    dst, w2p[:], AF.Copy, scale=invk[:, c : c + 1]
                                )
                    if dbg:
                        for c in range(EC):
                            w2d = dbgpool.tile([P, E], f32, name="w2_dbg")
                            nc.vector.tensor_copy(w2d[:], W2[c][:])
                            nc.sync.dma_start(dbg_w2[c * P : (c + 1) * P, :], w2d[:])

                # lag-2 software pipeline: posts trail pres by two blocks so
                # the W2 build and each block's inv-norm chain are covered by
                # ~30us of independent PE work
                assert NBLK >= 3
                xq_cur = xq0  # prefetched during phase A
                xq_nxt = load_xq(1)
                pres = [c_pre(0, xq_cur)]
                xq_cur, xq_nxt = xq_nxt, load_xq(2)
                pres.append(c_pre(1, xq_cur))
                emit_w2()
                for bi in range(2, NBLK):
                    xq_cur = xq_nxt
                    xq_nxt = load_xq(bi + 1) if bi + 1 < NBLK else None
                    pres.append(c_pre(bi, xq_cur))
                    c_post(bi - 2, *pres[bi - 2])
                    pres[bi - 2] = None
                c_post(NBLK - 2, *pres[NBLK - 2])
                c_post(NBLK - 1, *pres[NBLK - 1])
            w2scope.__exit__(None, None, None)
            c_in_scope.__exit__(None, None, None)

    _patch_bass(nc)
    return nc


# --------------------------------------------------------------------------
# Host wrapper
# --------------------------------------------------------------------------
_NC_CACHE = {}


def _get_nc(S, with_bias=False):
    key = (S, with_bias)
    if key not in _NC_CACHE:
        _NC_CACHE[key] = build(S, with_bias)
    return _NC_CACHE[key]


def _t_bf16(x):
    """[S, E] f32 -> [E, S] bf16 contiguous."""
    return np.ascontiguousarray(np.asarray(x, np.float32).astype(BF16).T)


def _bpick_const():
    """[H, EC*P]: slice ot is blockones_ot^T (per-head broadcast mask)."""
    bp = np.zeros((H, EC * P), np.float32)
    for ot in range(EC):
        bp[2 * ot, ot * P : ot * P + D] = 1.0
        bp[2 * ot + 1, ot * P + D : (ot + 1) * P] = 1.0
    return np.ascontiguousarray(bp.astype(BF16))


def make_in_maps(query, key, value, Wq, bq, Wk, bk, Wv, bv, Wo, bo):
    query = np.asarray(query, np.float32)
    B = query.shape[0]
    with_bias = any(np.any(np.asarray(b)) for b in (bq, bk, bv, bo))
    shared = {
        "WqT": _t_bf16(Wq),
        "WkT": _t_bf16(Wk),
        "WvT": _t_bf16(Wv),
        "WoT": _t_bf16(Wo),
        "bpick": _bpick_const(),
    }
    if with_bias:
        for name, b in (("bqr", bq), ("bkr", bk), ("bvr", bv), ("bor", bo)):
            shared[name] = np.ascontiguousarray(
                np.asarray(b, np.float32).reshape(1, E).astype(BF16)
            )
    return [
        {
            "xqT": _t_bf16(query[c]),
            "xkT": _t_bf16(key[c]),
            "xvT": _t_bf16(value[c]),
            **shared,
        }
        for c in range(B)
    ]


def kernel(query, key, value, Wq, bq, Wk, bk, Wv, bv, Wo, bo):
    query = np.asarray(query, np.float32)
    B, S, E_ = query.shape
    assert E_ == E and B == N_CORES
    in_maps = make_in_maps(query, key, value, Wq, bq, Wk, bk, Wv, bv, Wo, bo)
    with_bias = any(np.any(np.asarray(b)) for b in (bq, bk, bv, bo))
    nc = _get_nc(S, with_bias)
    res = run_bass_kernel_spmd(nc, in_maps, core_ids=list(range(N_CORES)))
    return np.stack([res.results[c]["out"] for c in range(B)])


# revision 76
# speedup vs baseline: 1.0086x; 1.0040x over previous
"""Trainium2 Bass kernel for nn_MultiHeadedLinrec (linear attention).

Math (per batch element, reference semantics):
    q = elu(x_q @ Wq.T + bq)    [S, E] viewed as [S, H, d]
    k = elu(x_k @ Wk.T + bk)
    v = x_v @ Wv.T + bv
    k <- k / (||k||_seq * sqrt(S))     (per (h, d) column norm over S)
    q <- q / (||q||_d   * sqrt(d))     (per (s, h) row norm over d)
    scores_h = k_h^T @ v_h             [d, d]
    out = concat_h(q_h @ scores_h) @ Wo.T + bo

Kernel strategy (one NeuronCore per batch element, 8 cores data-parallel):
  Host pre-transposes x_q/x_k/x_v to [E, S] and converts x + weights to
  bf16, so the device never runs PE transposes and every matmul streams
  bf16 moving operands at 1 cyc/row (fp32 PSUM accumulation throughout).

  Phase A (stream S in 512-col blocks, 128-row subtiles): project k and v
    into natural [s, e] layout (data chunks stationary, weight chunks
    moving), ELU(k), pack per-head [v|k] bf16 tiles, accumulate per-head
    scoresT = v_h^T k_h and the k Gram matrix on the PE (scores matmuls are
    emitted with a one-subtile lag so the in-order PE queue never waits on
    the ELU chain).
  Phase B: knorm from the Gram diagonal (DMA diag gather), then fold
    k-norm + scoresT + Wo into one fused weight
    W2[i, o] = (scores @ Wo.T)[i, o] / (knorm[i] * sqrt(S)), bf16.
  Phase C (stream S in 512-col blocks): q projection in transposed [e, s]
    layout (weight chunks stationary, data moving), ELU, per-head row
    norms via block-ones matmul + PE broadcast, then out = qn^T @ W2 + bo
    in natural layout.  B and C are software-pipelined: the W2 build and
    each block's inv-norm chain trail the projection stream by two blocks,
    so the in-order PE queue never waits on DVE/Act epilogues.  PSUM pools
    are created so late-needed pools take the banks phase A frees last.

This walrus build only supports ONE sync wait per instruction; Tile emits
multi-wait instructions, so we legalize the BIR JSON by hoisting extra waits
onto inserted NoOps (see _legalize_sync_json).
"""

import json

import ml_dtypes
import numpy as np

import concourse.bass as bass
import concourse.mybir as mybir
import concourse.tile as tile
from concourse.bass_utils import run_bass_kernel_spmd

dt = mybir.dt
AF = mybir.ActivationFunctionType
ALU = mybir.AluOpType

P = 128
E = 1024
H = 16
D = 64
N_CORES = 8
EC = E // P  # 8 chunks of 128 along the embedding dim
SBLK = 512  # s-block width
BF16 = ml_dtypes.bfloat16


# --------------------------------------------------------------------------
# BIR sync legalization: max one wait / one update per instruction.
# --------------------------------------------------------------------------
def _legalize_sync_json(bir_json: bytes) -> bytes:
    m = json.loads(bir_json)
    counter = [0]

    def fresh():
        counter[0] += 1
        return f"I-synclift-{counter[0]}"

    for f in m["functions"]:
        for blk in f["blocks"]:
            out = []
            for ins in blk["instructions"]:
                si = ins.get("sync_info")
                if not si:
                    out.append(ins)
                    continue
                waits = si.get("on_wait") or []
                updates = si.get("on_update") or []
                if len(waits) <= 1 and len(updates) <= 1:
                    out.append(ins)
                    continue
                eng = ins.get("engine")
                dbg = ins.get("debug")
                for w in waits[:-1]:
                    out.append(
                        {
                            "debug": dbg,
                            "engine": eng,
                            "ins": [],
                            "name": fresh(),
                            "opcode": "NoOp",
                            "outs": [],
                            "sync_info": {"on_update": [], "on_wait": [w]},
                        }
                    )
                si["on_wait"] = waits[-1:]
                post = [
                    {
                        "debug": dbg,
                        "engine": eng,
                        "ins": [],
                        "name": fresh(),
                        "opcode": "NoOp",
                        "outs": [],
                        "sync_info": {"on_update": [u], "on_wait": []},
                    }
                    for u in updates[1:]
                ]
                si["on_update"] = updates[:1]
                out.append(ins)
                out.extend(post)
            blk["instructions"] = out
    return json.dumps(m).encode()


def _patch_bass(nc):
    orig = nc.to_json_bytes

    def patched():
        return _legalize_sync_json(orig())

    nc.to_json_bytes = patched
    return nc


# --------------------------------------------------------------------------
# Kernel builder
# --------------------------------------------------------------------------
def build(S: int = 4096, with_bias: bool = False, cfg: dict | None = None):
    cfg = {"a_pj": 3, "a_kv": 3, "c_pj": 2, "c_fin": 3, "sc_lag": 2, "osb": "act", **(cfg or {})}
    dbg = cfg.get("debug", False)
    ST = S // P  # number of 128-row s-tiles
    NBLK = S // SBLK
    JB = SBLK // P  # s-tiles per block (4)

    nc = bass.Bass(trn_type="TRN2", target_bir_lowering=False, debug=False)

    bf = dt.bfloat16
    f32 = dt.float32

    xqT = nc.dram_tensor("xqT", [E, S], bf, kind="ExternalInput").ap()
    xkT = nc.dram_tensor("xkT", [E, S], bf, kind="ExternalInput").ap()
    xvT = nc.dram_tensor("xvT", [E, S], bf, kind="ExternalInput").ap()
    WqTd = nc.dram_tensor("WqT", [E, E], bf, kind="ExternalInput").ap()
    WkTd = nc.dram_tensor("WkT", [E, E], bf, kind="ExternalInput").ap()
    WvTd = nc.dram_tensor("WvT", [E, E], bf, kind="ExternalInput").ap()
    WoTd = nc.dram_tensor("WoT", [E, E], bf, kind="ExternalInput").ap()
    if with_bias:
        bqr = nc.dram_tensor("bqr", [1, E], bf, kind="ExternalInput").ap()
        bkr = nc.dram_tensor("bkr", [1, E], bf, kind="ExternalInput").ap()
        bvr = nc.dram_tensor("bvr", [1, E], bf, kind="ExternalInput").ap()
        bor = nc.dram_tensor("bor", [1, E], bf, kind="ExternalInput").ap()
    bpickd = nc.dram_tensor("bpick", [H, EC * P], bf, kind="ExternalInput").ap()
    out = nc.dram_tensor("out", [S, E], f32, kind="ExternalOutput").ap()
    if dbg:
        dbg_kv = nc.dram_tensor("dbg_kv", [P, 2 * E], f32, kind="ExternalOutput").ap()
        dbg_gram = nc.dram_tensor("dbg_gram", [D, H * D], f32, kind="ExternalOutput").ap()
        dbg_invk = nc.dram_tensor("dbg_invk", [P, EC], f32, kind="ExternalOutput").ap()
        dbg_w2 = nc.dram_tensor("dbg_w2", [E, E], f32, kind="ExternalOutput").ap()
        dbg_qt = nc.dram_tensor("dbg_qt", [E, SBLK], f32, kind="ExternalOutput").ap()
        dbg_qn = nc.dram_tensor("dbg_qn", [E, SBLK], f32, kind="ExternalOutput").ap()

    with tile.TileContext(nc) as tc:
        with (
            tc.tile_pool(name="consts", bufs=1) as consts,
            tc.tile_pool(name="small", bufs=1) as small,
            tc.tile_pool(name="wts", bufs=1) as wts,
            tc.tile_pool(name="drpool", bufs=1, space="DRAM") as drpool,
            tc.tile_pool(name="dbgpool", bufs=1) as dbgpool,
        ):
            # ---------------- constants ----------------
            # zrow first: the scores-psum zeroing matmuls are the PE's first
            # instructions and should not wait on the other const memsets
            zrow = consts.tile([1, SBLK], bf, name="zrow")
            nc.vector.memset(zrow[:], 0.0)
            # blockones_ot [P, H]: col 2*ot -> partitions 0:64, col 2*ot+1 ->
            # partitions 64:128 (head-pair sum masks for qss accumulation)
            blockones = []
            for c in range(EC):
                t = consts.tile([P, H], bf, name=f"blockones_{c}")
                nc.vector.memset(t[:], 0.0)
                nc.vector.memset(t[0:D, 2 * c : 2 * c + 1], 1.0)
                nc.vector.memset(t[D:P, 2 * c + 1 : 2 * c + 2], 1.0)
                blockones.append(t)
            # blockpick_ot [H, P] = blockones_ot^T (broadcast masks); loaded
            # from a host constant because DVE memsets cannot start at odd
            # partitions.
            bpick_all = consts.tile([H, EC * P], bf, name="bpick_all")
            nc.sync.dma_start(bpick_all[:], bpickd)
            blockpick = [bpick_all[:, c * P : (c + 1) * P] for c in range(EC)]
            ones_row = None
            if with_bias:
                ones_row = consts.tile([1, SBLK], bf, name="ones_row")
                nc.vector.memset(ones_row[:], 1.0)

            # bd staging (scoresT block-diagonal per chunk), zeroed once
            bd_st = []
            for pr in range(EC):
                s_t = small.tile([P, P], f32, name=f"bd_st_{pr}")
                nc.vector.memset(s_t[:], 0.0)
                bd_st.append(s_t)

            # ---------------- weights (all bf16, straight DMA) -----------
            def load_wt(WTd, name):
                tiles = []
                for c in range(EC):
                    t = wts.tile([P, E], bf, name=f"{name}T_{c}")
                    nc.sync.dma_start(t[:], WTd[c * P : (c + 1) * P, :])
                    tiles.append(t)
                return tiles

            def load_brow(src, name):
                t = small.tile([1, E], bf, name=name)
                nc.sync.dma_start(t[:], src)
                return t

            bk_row = bv_row = bq_row = bo_row = None
            if with_bias:
                bk_row = load_brow(bkr, "bk_row")
                bv_row = load_brow(bvr, "bv_row")

            # ================= PHASE A ====================================
            c_in_scope = tc.tile_pool(name="c_in", bufs=2)
            c_in = c_in_scope.__enter__()
            a_in_scope = tc.tile_pool(name="a_in", bufs=2)
            a_in = a_in_scope.__enter__()

            def load_xq(bi):
                t = c_in.tile([P, EC * SBLK], bf, name="xq_blk")
                nc.sync.dma_start(
                    t[:].rearrange("p (c s) -> p c s", c=EC),
                    xqT[:, bi * SBLK : (bi + 1) * SBLK].rearrange(
                        "(c p) s -> p c s", p=P
                    ),
                )
                return t

            def load_xblk(src, bi, name):
                t = a_in.tile([P, EC * SBLK], bf, name=name)
                nc.sync.dma_start(
                    t[:].rearrange("p (c s) -> p c s", c=EC),
                    src[:, bi * SBLK : (bi + 1) * SBLK].rearrange(
                        "(c p) s -> p c s", p=P
                    ),
                )
                return t

            # Startup: interleave the k-weight and first-block loads per
            # chunk so the first projection matmuls start ~1us in (the DMA
            # device is serial; a monolithic 3MB prefix stalls the PE 12us)
            WkT = [wts.tile([P, E], bf, name=f"WkT_{c}") for c in range(EC)]
            WvT = [wts.tile([P, E], bf, name=f"WvT_{c}") for c in range(EC)]
            xk_cur = a_in.tile([P, EC * SBLK], bf, name="xk_blk")
            xv_cur = a_in.tile([P, EC * SBLK], bf, name="xv_blk")
            for c in range(EC):
                nc.sync.dma_start(WkT[c][:], WkTd[c * P : (c + 1) * P, :])
                nc.sync.dma_start(
                    xk_cur[:, c * SBLK : (c + 1) * SBLK],
                    xkT[c * P : (c + 1) * P, 0:SBLK],
                )
            for c in range(EC):
                nc.sync.dma_start(WvT[c][:], WvTd[c * P : (c + 1) * P, :])
                nc.sync.dma_start(
                    xv_cur[:, c * SBLK : (c + 1) * SBLK],
                    xvT[c * P : (c + 1) * P, 0:SBLK],
                )
            WqT = load_wt(WqTd, "Wq")
            WoT = load_wt(WoTd, "Wo")
            if with_bias:
                bq_row = load_brow(bqr, "bq_row")
                bo_row = load_brow(bor, "bo_row")

            a_sc_scope = tc.tile_pool(name="a_sc_ps", bufs=1, space="PSUM")
            a_sc_ps = a_sc_scope.__enter__()
            scores_ps = a_sc_ps.tile([P, H * D], f32, name="scores_ps")
            # Pre-zero via start=True matmuls: a matmul's start bit zeroes the
            # whole PSUM bank, so the per-head 64-col accumulation below must
            # never use start=True (it would erase sibling heads' partials).
            for hb in range(2):
                nc.tensor.matmul(
                    scores_ps[:, hb * SBLK : (hb + 1) * SBLK],
                    zrow[:, 0:P],
                    zrow[:],
                    start=True,
                    stop=True,
                    skip_group_check=True,
                )

            with (
                tc.tile_pool(name="a_pj_ps", bufs=cfg["a_pj"] * (2 if cfg.get("a_shared") else 1), space="PSUM") as a_pj_ps,
                tc.tile_pool(name="a_kv", bufs=cfg["a_kv"]) as a_kv,
                tc.tile_pool(name="a_tmp", bufs=3) as a_tmp,
            ):
                def project(xblk, WT, brow, j, name):
                    """[s=128, o=512] psum halves of a projection.

                    xblk free layout is (c, s): col = c*SBLK + s.
                    """
                    halves = []
                    for h in range(2):
                        pj = a_pj_ps.tile([P, SBLK], f32, name="pj" if cfg.get("a_shared") else f"{name}_pj")
                        for c in range(EC):
                            nc.tensor.matmul(
                                pj[:],
                                xblk[:, c * SBLK + j * P : c * SBLK + (j + 1) * P],
                                WT[c][:, h * SBLK : (h + 1) * SBLK],
                                start=(c == 0),
                                stop=(brow is None and c == EC - 1),
                            )
                        if brow is not None:
                            nc.tensor.matmul(
                                pj[:],
                                ones_row[:, 0:P],
                                brow[:, h * SBLK : (h + 1) * SBLK],
                                start=False,
                                stop=True,
                            )
                        halves.append(pj)
                    return halves

                def emit_scores(kv_sb, gj):
                    for hh in range(H):
                        nc.tensor.matmul(
                            scores_ps[:, hh * D : (hh + 1) * D],
                            kv_sb[:, 2 * D * hh : 2 * D * (hh + 1)],
                            kv_sb[:, 2 * D * hh + D : 2 * D * (hh + 1)],
                            start=False,
                            stop=(gj == ST - 1 and hh == H - 1),
                            skip_group_check=True,
                        )

                kv_fifo = []  # (kv_sb, global_subtile), lag cfg["sc_lag"]
                sc_lag = cfg.get("sc_lag", 1)
                xq0 = None
                for bi in range(NBLK):
                    xk_nxt = xv_nxt = None
                    if bi + 1 < NBLK:
                        xk_nxt = load_xblk(xkT, bi + 1, "xk_blk")
                        xv_nxt = load_xblk(xvT, bi + 1, "xv_blk")
                    if bi == NBLK - 1:
                        # prefetch the first q block so phase C starts hot
                        xq0 = load_xq(0)
                    for j in range(JB):
                        gj = bi * JB + j
                        # per-head interleave: head hh cols [128*hh, 128*hh+128)
                        # v in the low 64, elu(k) in the high 64
                        kv_sb = a_kv.tile([P, 2 * E], bf, name="kv_sb")
                        kv4 = kv_sb[:].rearrange(
                            "p (hh two) -> p hh two", two=2 * D
                        )
                        kp = project(xk_cur, WkT, bk_row, j, "k")
                        for h in range(2):
                            r_sb = a_tmp.tile([P, SBLK], bf, name="kr_sb")
                            t_sb = a_tmp.tile([P, SBLK], bf, name="kt_sb")
                            e_sb = a_tmp.tile([P, SBLK], bf, name="ke_sb")
                            # relu on DVE to balance the Act engine (Exp+copies)
                            nc.vector.tensor_scalar(
                                r_sb[:], kp[h][:], 0.0, None, ALU.max
                            )
                            # elu(x) = relu(x) + min(exp(x), 1) - 1
                            nc.scalar.activation(e_sb[:], kp[h][:], AF.Exp)
                            nc.vector.tensor_scalar(
                                t_sb[:], e_sb[:], 1.0, -1.0, ALU.min, ALU.add
                            )
                            nc.gpsimd.tensor_tensor(
                                kv4[:, 8 * h : 8 * (h + 1), D : 2 * D],
                                t_sb[:].rearrange("p (hh d) -> p hh d", d=D),
                                r_sb[:].rearrange("p (hh d) -> p hh d", d=D),
                                ALU.add,
                            )
                        vp = project(xv_cur, WvT, bv_row, j, "v")
                        for h in range(2):
                            nc.scalar.copy(
                                kv4[:, 8 * h : 8 * (h + 1), 0:D],
                                vp[h][:].rearrange("p (hh d) -> p hh d", d=D),
                            )
                        if dbg and gj == 0:
                            kvd = dbgpool.tile([P, 2 * E], f32, name="kv_dbg")
                            nc.vector.tensor_copy(kvd[:], kv_sb[:])
                            nc.sync.dma_start(dbg_kv, kvd[:])
                        # scores lag so the PE never waits on the ELU chain
                        kv_fifo.append((kv_sb, gj))
                        if len(kv_fifo) > sc_lag:
                            emit_scores(*kv_fifo.pop(0))
                    xk_cur, xv_cur = xk_nxt, xv_nxt
                for ent in kv_fifo:
                    emit_scores(*ent)

                # -- extract scoresT + ksumsq while phase-A psum still alive
                # Gram rows (64:128) hold k^T k per head; diagonal = ksumsq
                gram_sb = small.tile([D, H * D], f32, name="gram_sb")
                nc.scalar.copy(gram_sb[:], scores_ps[D:P, :])
                gram_dram = drpool.tile([1, D * H * D], f32, name="gram_dram")
                nc.sync.dma_start(
                    gram_dram[:].rearrange("1 (d c) -> d c", d=D), gram_sb[:]
                )
                # diag idx for (hh, d) = d*(H*D) + hh*D + d = d*(H*D+1) + D*hh
                kcol = small.tile([P, EC], f32, name="kcol")
                gd = gram_dram[:].tensor
                for h2 in range(2):
                    src_ap = bass.AP(gd, h2 * D, [[H * D + 1, D], [2 * D, EC]])
                    nc.sync.dma_start(kcol[h2 * D : (h2 + 1) * D, :], src_ap)
                knorm = small.tile([P, EC], f32, name="knorm")
                nc.scalar.activation(knorm[:], kcol[:], AF.Sqrt, scale=float(S))
                invk = small.tile([P, EC], f32, name="invk")
                nc.vector.reciprocal(invk[:], knorm[:])
                if dbg:
                    nc.sync.dma_start(dbg_gram, gram_sb[:])
                    nc.sync.dma_start(dbg_invk, invk[:])

                bd = []
                for pr in range(EC):
                    h0, h1 = 2 * pr, 2 * pr + 1
                    nc.vector.tensor_copy(
                        bd_st[pr][0:D, 0:D], scores_ps[0:D, h0 * D : (h0 + 1) * D]
                    )
                    odd_stage = small.tile([D, D], f32, name="odd_stage")
                    nc.vector.tensor_copy(
                        odd_stage[:], scores_ps[0:D, h1 * D : (h1 + 1) * D]
                    )
                    nc.sync.dma_start(bd_st[pr][D:P, D:P], odd_stage[:])
                    bd_t = small.tile([P, P], bf, name=f"bd_{pr}")
                    nc.gpsimd.tensor_copy(bd_t[:], bd_st[pr][:])
                    bd.append(bd_t)

            a_sc_scope.__exit__(None, None, None)
            a_in_scope.__exit__(None, None, None)

            # ================= PHASES B + C, software-pipelined ===========
            # Emission order: C-pre(0) | B (W2 build) | C-pre(1) C-post(0) |
            # C-pre(2) C-post(1) | ... — the W2 build and each block's
            # inv-norm chain overlap the next block's projection matmuls, so
            # the PE never waits on the DVE/Act epilogues.
            w2scope = tc.tile_pool(name="w2pool", bufs=1)
            w2pool = w2scope.__enter__()
            W2 = [w2pool.tile([P, E], bf, name=f"W2_{c}") for c in range(EC)]
            with (
                tc.tile_pool(name="c_qt", bufs=4) as c_qt,
                tc.tile_pool(name="c_nrm", bufs=4) as c_nrm,
                tc.tile_pool(name="c_qn", bufs=1) as c_qn,
                tc.tile_pool(name="c_q2", bufs=1) as c_q2,
                tc.tile_pool(name="c_tmp", bufs=3) as c_tmp,
                tc.tile_pool(name="c_out", bufs=2) as c_out,
                # PSUM is a stack allocator: pools created first take the
                # lowest banks, which phase A's scores pool just vacated and
                # which only free after the Gram/bd epilogue reads.  Put the
                # late-needed fin/qb pools there; q_pj (needed immediately)
                # then lands on the early-freed projection banks.
                tc.tile_pool(name="c_fin_ps", bufs=cfg["c_fin"], space="PSUM") as c_fin_ps,
                tc.tile_pool(name="c_qb_ps", bufs=1, space="PSUM") as c_qb_ps,
                tc.tile_pool(name="c_ss_ps", bufs=2, space="PSUM") as c_ss_ps,
                tc.tile_pool(name="c_pj_ps", bufs=cfg["c_pj"], space="PSUM") as c_pj_ps,
            ):
                def c_pre(bi, xq_cur):
                    """q projection + ELU + compact row-norm for block bi.

                    The norm uses the [H, SBLK] compact form so the DVE
                    reciprocal (~6 HW cycles/element) runs once per block.
                    """
                    qt_tiles = []
                    q2_tiles = []
                    for ot in range(EC):
                        pj = c_pj_ps.tile([P, SBLK], f32, name="q_pj")
                        for c in range(EC):
                            nc.tensor.matmul(
                                pj[:],
                                WqT[c][:, ot * P : (ot + 1) * P],
                                xq_cur[:, c * SBLK : (c + 1) * SBLK],
                                start=(c == 0),
                                stop=(bq_row is None and c == EC - 1),
                            )
                        if bq_row is not None:
                            nc.tensor.matmul(
                                pj[:],
                                bq_row[:, ot * P : (ot + 1) * P],
                                ones_row[:],
                                start=False,
                                stop=True,
                            )
                        r_sb = c_tmp.tile([P, SBLK], bf, name="qr_sb")
                        t_sb = c_tmp.tile([P, SBLK], bf, name="qt_sb")
                        e_sb = c_tmp.tile([P, SBLK], bf, name="qe_sb")
                        qt_ = c_qt.tile([P, SBLK], bf, name=f"qt_{ot}")
                        nc.scalar.activation(r_sb[:], pj[:], AF.Relu)
                        # elu(x) = relu(x) + min(exp(x), 1) - 1
                        nc.scalar.activation(e_sb[:], pj[:], AF.Exp)
                        nc.vector.tensor_scalar(
                            t_sb[:], e_sb[:], 1.0, -1.0, ALU.min, ALU.add
                        )
                        nc.vector.tensor_tensor(qt_[:], t_sb[:], r_sb[:], ALU.add)
                        qt_tiles.append(qt_)
                        q2 = c_q2.tile([P, SBLK], bf, name=f"q2_{ot}")
                        # SBUF-only square on the idle GPSIMD engine
                        nc.gpsimd.tensor_tensor(q2[:], qt_[:], qt_[:], ALU.mult)
                        q2_tiles.append(q2)
                    qss_ps = c_ss_ps.tile([H, SBLK], f32, name="qss_ps")
                    for ot in range(EC):
                        nc.tensor.matmul(
                            qss_ps[:],
                            blockones[ot][:],
                            q2_tiles[ot][:],
                            start=(ot == 0),
                            stop=(ot == EC - 1),
                        )
                    # invq = 1 / sqrt(D * qss), emitted bf16 for PE broadcast
                    # (Sqrt first, on moderate-magnitude inputs: the Act-engine
                    # Sqrt table is inaccurate for tiny inputs)
                    qss_sb = c_nrm.tile([H, SBLK], f32, name="qss_sb")
                    nc.scalar.activation(
                        qss_sb[:], qss_ps[:], AF.Sqrt, scale=float(D)
                    )
                    invq = c_nrm.tile([H, SBLK], f32, name="invq")
                    nc.vector.reciprocal(invq[:], qss_sb[:])
                    invq_r = c_nrm.tile([H, SBLK], bf, name="invq_r")
                    nc.vector.tensor_copy(invq_r[:], invq[:])
                    if dbg and bi == 0:
                        for ot in range(EC):
                            qtd = dbgpool.tile([P, SBLK], f32, name="qt_dbg")
                            nc.vector.tensor_copy(qtd[:], qt_tiles[ot][:])
                            nc.sync.dma_start(
                                dbg_qt[ot * P : (ot + 1) * P, :], qtd[:]
                            )
                    return (qt_tiles, invq_r)

                def c_post(bi, qt_tiles, invq_r):
                    """inv-norm broadcast, q scaling, fused output GEMM."""
                    s0 = bi * SBLK
                    qn_tiles = []
                    for ot in range(EC):
                        qb = c_qb_ps.tile([P, SBLK], f32, name="qb_ps")
                        nc.tensor.matmul(
                            qb[:], blockpick[ot], invq_r[:],
                            start=True, stop=True,
                        )
                        qn = c_qn.tile([P, SBLK], bf, name=f"qn_{ot}")
                        nc.vector.tensor_tensor(
                            qn[:], qt_tiles[ot][:], qb[:], ALU.mult
                        )
                        if dbg and bi == 0:
                            qnd = dbgpool.tile([P, SBLK], f32, name="qn_dbg")
                            nc.vector.tensor_copy(qnd[:], qn[:])
                            nc.sync.dma_start(
                                dbg_qn[ot * P : (ot + 1) * P, :], qnd[:]
                            )
                        qn_tiles.append(qn)
                    for j2 in range(JB // 2):
                        o_sb = c_out.tile([P, 2 * E], f32, name="o_sb")
                        for tj in range(2):
                            j = j2 * 2 + tj
                            for h in range(2):
                                fin = c_fin_ps.tile([P, SBLK], f32, name="fin_ps")
                                for c in range(EC):
                                    nc.tensor.matmul(
                                        fin[:],
                                        qn_tiles[c][:, j * P : (j + 1) * P],
                                        W2[c][:, h * SBLK : (h + 1) * SBLK],
                                        start=(c == 0),
                                        stop=(bo_row is None and c == EC - 1),
                                    )
                                if bo_row is not None:
                                    nc.tensor.matmul(
                                        fin[:],
                                        ones_row[:, 0:P],
                                        bo_row[:, h * SBLK : (h + 1) * SBLK],
                                        start=False,
                                        stop=True,
                                    )
                                sl = slice(tj * E + h * SBLK, tj * E + (h + 1) * SBLK)
                                osb_mode = cfg.get("osb", "alt")
                                if osb_mode == "act" or (
                                    osb_mode == "alt" and (j + h) % 2 == 1
                                ):
                                    nc.scalar.copy(o_sb[:, sl], fin[:])
                                else:
                                    nc.vector.tensor_copy(o_sb[:, sl], fin[:])
                            # store per 128-row tile so the final store isn't
                            # serialized behind both tiles' copies
                            nc.sync.dma_start(
                                out[s0 + j * P : s0 + (j + 1) * P, :],
                                o_sb[:, tj * E : (tj + 1) * E],
                            )

                def emit_w2():
                    """W2 = knorm^-1 * scoresT @ WoT; psums share the fin ring."""
                    for c in range(EC):
                        for h in range(2):
                            w2p = c_fin_ps.tile([P, SBLK], f32, name="fin_ps")
                            nc.tensor.matmul(
                                w2p[:],
                                bd[c][:],
                                WoT[c][:, h * SBLK : (h + 1) * SBLK],
                                start=True,
                                stop=True,
                            )
                            dst = W2[c][:, h * SBLK : (h + 1) * SBLK]
                            if (c + h) % 2 == 0:
                                nc.vector.tensor_scalar(
                                    dst, w2p[:], invk[:, c : c + 1], None, ALU.mult
                                )
                            else:
                                nc.scalar.activation(
                                    dst, w2p[:], AF.Copy, scale=invk[:, c : c + 1]
                                )
                    if dbg:
                        for c in range(EC):
                            w2d = dbgpool.tile([P, E], f32, name="w2_dbg")
                            nc.vector.tensor_copy(w2d[:], W2[c][:])
                            nc.sync.dma_start(dbg_w2[c * P : (c + 1) * P, :], w2d[:])

                # lag-2 software pipeline: posts trail pres by two blocks so
                # the W2 build and each block's inv-norm chain are covered by
                # ~30us of independent PE work
                assert NBLK >= 3
                xq_cur = xq0  # prefetched during phase A
                xq_nxt = load_xq(1)
                pres = [c_pre(0, xq_cur)]
                xq_cur, xq_nxt = xq_nxt, load_xq(2)
                pres.append(c_pre(1, xq_cur))
                emit_w2()
                for bi in range(2, NBLK):
                    xq_cur = xq_nxt
                    xq_nxt = load_xq(bi + 1) if bi + 1 < NBLK else None
                    pres.append(c_pre(bi, xq_cur))
                    c_post(bi - 2, *pres[bi - 2])
                    pres[bi - 2] = None
                c_post(NBLK - 2, *pres[NBLK - 2])
                c_post(NBLK - 1, *pres[NBLK - 1])
            w2scope.__exit__(None, None, None)
            c_in_scope.__exit__(None, None, None)

    _patch_bass(nc)
    return nc


# --------------------------------------------------------------------------
# Host wrapper
# --------------------------------------------------------------------------
_NC_CACHE = {}


def _get_nc(S, with_bias=False):
    key = (S, with_bias)
    if key not in _NC_CACHE:
        _NC_CACHE[key] = build(S, with_bias)
    return _NC_CACHE[key]


def _t_bf16(x):
    """[S, E] f32 -> [E, S] bf16 contiguous."""
    return np.ascontiguousarray(np.asarray(x, np.float32).astype(BF16).T)


def _bpick_const():
    """[H, EC*P]: slice ot is blockones_ot^T (per-head broadcast mask)."""
    bp = np.zeros((H, EC * P), np.float32)
    for ot in range(EC):
        bp[2 * ot, ot * P : ot * P + D] = 1.0
        bp[2 * ot + 1, ot * P + D : (ot + 1) * P] = 1.0
    return np.ascontiguousarray(bp.astype(BF16))


def make_in_maps(query, key, value, Wq, bq, Wk, bk, Wv, bv, Wo, bo):
    query = np.asarray(query, np.float32)
    B = query.shape[0]
    with_bias = any(np.any(np.asarray(b)) for b in (bq, bk, bv, bo))
    shared = {
        "WqT": _t_bf16(Wq),
        "WkT": _t_bf16(Wk),
        "WvT": _t_bf16(Wv),
        "WoT": _t_bf16(Wo),
        "bpick": _bpick_const(),
    }
    if with_bias:
        for name, b in (("bqr", bq), ("bkr", bk), ("bvr", bv), ("bor", bo)):
            shared[name] = np.ascontiguousarray(
                np.asarray(b, np.float32).reshape(1, E).astype(BF16)
            )
    return [
        {
            "xqT": _t_bf16(query[c]),
            "xkT": _t_bf16(key[c]),
            "xvT": _t_bf16(value[c]),
            **shared,
        }
        for c in range(B)
    ]


def kernel(query, key, value, Wq, bq, Wk, bk, Wv, bv, Wo, bo):
    query = np.asarray(query, np.float32)
    B, S, E_ = query.shape
    assert E_ == E and B == N_CORES
    in_maps = make_in_maps(query, key, value, Wq, bq, Wk, bk, Wv, bv, Wo, bo)
    with_bias = any(np.any(np.asarray(b)) for b in (bq, bk, bv, bo))
    nc = _get_nc(S, with_bias)
    res = run_bass_kernel_spmd(nc, in_maps, core_ids=list(range(N_CORES)))
    return np.stack([res.results[c]["out"] for c in range(B)])
